# revision 16
# baseline (speedup 1.0000x reference)
"""Trainium2 Bass kernel for 16-head causal attention (transposed-softmax variant).

Problem shapes: x [8, 1024, 1024]; W_K/W_Q/W_V [16, 64, 1024]; W_O [1024, 1024].
Sharding: pure data-parallel over batch (8 batch elements -> 8 cores), weights
replicated, no collectives.

Per-core pipeline (one batch element, seq=1024, d_embed=1024, 16 heads x 64):
  1. QKV projections as K_T/Q_T [heads*64, seq] and V [seq, heads*64], fp16
     operands, fp32 PSUM accumulation. W_Q is pre-scaled by 1/sqrt(d_head) on
     the host so scores come out pre-scaled.
  2. Per head pair: scores S[c, C] = K[c].Q[C] for causal-allowed C-chunks
     only, the two heads' K=64 matmuls interleaved so they run concurrently in
     disjoint PE row-groups.
  3. Softmax over C without max-subtraction (scores ~ N(0,1)). Row-tiles
     i<=5: exp on the scalar engine, then one fused vector-engine
     scalar_tensor_tensor per (head, tile): E *= m1-slice (ones over the
     strict prefix, lower-triangular 0/1 over the diagonal block) in place,
     with accum_out = causal row sum. Row-tiles i in {6, 7} (the widest, most
     expensive on the vector engine): additive NEG mask on the diagonal block
     (vector-engine add into PSUM), then exp WITH accum_out on the scalar
     engine — splitting the row-sum work across both engines. Reciprocals are
     batched per head ([128, 8] in one op); V rows are scaled by 1/rowsum with
     one broadcast tensor_tensor per head.
  4. Z^T[h, C] += V'^T E per c-tile (descending i, N trimmed to the causal
     prefix, two heads col-split in one PSUM bank); output projection
     Z_flat @ W_O^T. For output row-blocks m 0-3, the first six
     f-contraction steps run early (inside pairs 6/7, which are otherwise
     PE-lean) into an fp16 partial; the tail only finishes p2 in {6,7} and
     adds the partial, shortening the serial output phase.

Scheduling: the scalar engine carries ~115us of exp(+accum) and the PE
~145us of matmul; dense projection groups spread as fillers across ALL eight
pairs so the PE never starves and the HAM clock gate stays at full rate.
kq weights for pair p+1 cc=0 are emitted inside pair p; the cc=1 half is
emitted inside pair p+1's first rows (it is only needed from row 4 on),
which starts the first exp several microseconds earlier. Each pair's AV
block interleaves into the next pair's rows.
"""

import numpy as np

S, E, A, H, B = 1024, 1024, 16, 64, 8
P = 128          # partitions
NEG = -30000.0   # additive mask value (fp16-safe; exp -> 0 in fp32)

_cache = {}


def _off(i):
    """Compact E-buffer offset of row-tile i (valid width of row i is (i+1)*P)."""
    return P * i * (i + 1) // 2


EW = _off(8)     # 4608 columns total

ACT_ACCUM_TILES = (6, 7)   # row-tiles whose row sum rides the scalar engine


def _build_nc():
    import concourse.bass as bass
    import concourse.mybir as mybir
    from concourse.tile import TileContext

    f16 = mybir.dt.float16
    f32 = mybir.dt.float32
    Exp = mybir.ActivationFunctionType.Exp
    mult = mybir.AluOpType.mult
    add = mybir.AluOpType.add

    nc = bass.Bass()
    xt_d = nc.dram_tensor("xt", [E, S], f16, kind="ExternalInput")        # x[b].T
    wkqv_d = nc.dram_tensor("wkqv", [E, 3 * A * H], f16, kind="ExternalInput")
    wo_d = nc.dram_tensor("wo", [A * H, E], f16, kind="ExternalInput")    # W_O.T
    aux_d = nc.dram_tensor("aux", [P, P + (S + P) + P], f16,
                           kind="ExternalInput")  # ident | m1 | mskn packed
    out_d = nc.dram_tensor("out", [S, E], f16, kind="ExternalOutput")

    ET = E // P       # 8 e-tiles
    CT = S // P       # 8 c-tiles
    NC = S // 512     # 2 512-chunks

    with TileContext(nc) as tc:
        with (
            tc.tile_pool(name="inp", bufs=1) as inp,
            tc.tile_pool(name="kqv", bufs=1) as kqv,
            tc.tile_pool(name="epool", bufs=4) as epool,
            tc.tile_pool(name="stats", bufs=12) as stats,
            tc.tile_pool(name="outp", bufs=3) as outp,
            tc.tile_pool(name="psq", bufs=4, space="PSUM") as psq,
            tc.tile_pool(name="pss", bufs=2, space="PSUM") as pss,
        ):
            # ---- SBUF destinations ----
            xT = inp.tile([P, ET, S], f16, tag="xT")
            wkqv = inp.tile([P, ET, 3 * A * H], f16, tag="wkqv")
            wo = inp.tile([P, ET, E], f16, tag="wo")
            aux = inp.tile([P, P + (S + P) + P], f16, tag="aux")
            wpart = inp.tile([P, 4, NC, 512], f16, tag="wpart")
            ident = aux[:, 0:P]
            m1 = aux[:, P:P + S + P]
            mskn = aux[:, P + S + P:P + S + 2 * P]

            # ---- loads: medium-grained transfers so data lands progressively
            # (fine deps for the first kq groups), split across three DMA
            # queues (sync + scalar HWDGE, gpsimd SWDGE) to bound the ~0.6us
            # per-dma sequencer issue cost on any one queue ----
            xr = xt_d[:].rearrange("(t p) c -> p t c", p=P)      # [128, 8, 1024]
            wr = wkqv_d[:].rearrange("(t p) f -> p t f", p=P)    # [128, 8, 3072]
            wor = wo_d[:].rearrange("(t p) f -> p t f", p=P)
            nc.sync.dma_start(aux[:], aux_d[:])
            for t2 in range(0, ET, 2):   # pair-0 K+Q cols and first x half
                sl = slice(t2, t2 + 2)
                nc.sync.dma_start(wkqv[:, sl, 0:P], wr[:, sl, 0:P])
                nc.sync.dma_start(wkqv[:, sl, A * H:A * H + P],
                                  wr[:, sl, A * H:A * H + P])
                nc.sync.dma_start(xT[:, sl, 0:512], xr[:, sl, 0:512])
            for t4 in range(0, ET, 4):   # second x half, then K rest
                sl = slice(t4, t4 + 4)
                nc.sync.dma_start(xT[:, sl, 512:S], xr[:, sl, 512:S])
            for t4 in range(0, ET, 4):
                sl = slice(t4, t4 + 4)
                nc.sync.dma_start(wkqv[:, sl, P:A * H], wr[:, sl, P:A * H])
            nc.scalar.dma_start(wkqv[:, :, 2 * A * H:2 * A * H + 512],
                                wr[:, :, 2 * A * H:2 * A * H + 512])         # V fc0
            nc.scalar.dma_start(wkqv[:, :, A * H + P:2 * A * H],
                                wr[:, :, A * H + P:2 * A * H])               # Q rest
            nc.gpsimd.dma_start(wkqv[:, :, 2 * A * H + 512:3 * A * H],
                                wr[:, :, 2 * A * H + 512:3 * A * H])         # V fc1
            nc.gpsimd.dma_start(wo[:, :, :], wor[:, :, :])

            K_T = kqv.tile([P, A // 2, S], f16, tag="K_T")   # pair-stacked [2h, c]
            Q_T = kqv.tile([P, A // 2, S], f16, tag="Q_T")
            V = kqv.tile([P, CT, A * H], f16, tag="V")       # [c, f]
            zT = kqv.tile([P, A // 2, S], f16, tag="zT")     # pair-stacked [f, C]

            # ---- PE warm-up: ride out the HAM throttle during the DMA wait ----
            wps = psq.tile([P, 512], f32, tag="psq", name="warm")
            for w in range(48):
                nc.tensor.matmul(wps[:, :P], ident[:], ident[:],
                                 start=(w == 0), stop=(w == 47),
                                 skip_group_check=True)
            wsb = stats.tile([P, 1], f32, tag="ssum", name="warmsink")
            nc.vector.reduce_max(wsb[:], wps[:, :P], axis=mybir.AxisListType.X)

            # ---- dense projection groups (used as attention fillers) ----
            def kq_group(p, mat, cc, cast_eng="scalar"):
                dst = K_T if mat == 0 else Q_T
                ps = psq.tile([P, 512], f32, tag="psq", name=f"q{p}{mat}{cc}")
                for et in range(ET):
                    nc.tensor.matmul(
                        ps[:],
                        wkqv[:, et, mat * A * H + p * P: mat * A * H + (p + 1) * P],
                        xT[:, et, cc * 512:(cc + 1) * 512],
                        start=(et == 0), stop=(et == ET - 1),
                    )
                if cast_eng == "scalar":
                    nc.scalar.copy(out=dst[:, p, cc * 512:(cc + 1) * 512], in_=ps[:])
                else:
                    nc.vector.tensor_copy(out=dst[:, p, cc * 512:(cc + 1) * 512],
                                          in_=ps[:])

            def v_group(fc, i):
                ps = psq.tile([P, 512], f32, tag="psq", name=f"v{fc}{i}")
                for et in range(ET):
                    nc.tensor.matmul(
                        ps[:],
                        xT[:, et, i * P:(i + 1) * P],
                        wkqv[:, et, 2 * A * H + fc * 512: 2 * A * H + (fc + 1) * 512],
                        start=(et == 0), stop=(et == ET - 1),
                    )
                nc.vector.tensor_copy(out=V[:, i, fc * 512:(fc + 1) * 512], in_=ps[:])

            def kq_cc(p, cc, cast_eng="scalar"):
                return [lambda mat=mat: kq_group(p, mat, cc, cast_eng)
                        for mat in (0, 1)]

            def v_closures(fc):
                return [lambda fc=fc, i=i: v_group(fc, i) for i in range(CT)]

            # ---- attention ----
            def attn_rows(p, early, fillers):
                """Scores+exp+mask/rowsum rows of pair p. `early` closures run
                in rows 0-3 (cc=1 kq groups, needed from row 4); `fillers`
                interleave across all rows. Returns (heads, E tiles)."""
                heads = [(2 * p, 0), (2 * p + 1, H)]
                Ets = [epool.tile([P, EW], f16, tag="E", name=f"E{k}_{p}")
                       for k in range(2)]
                sums = [stats.tile([P, CT], f32, tag="ssum", name=f"sm{k}_{p}")
                        for k in range(2)]
                rcps = [stats.tile([P, CT], f32, tag="rcp", name=f"rc{k}_{p}")
                        for k in range(2)]
                fq = list(fillers)
                eq = list(early)
                w = [2, 2, 2, 2, 3, 3, 3, 3]
                tot = sum(w)
                share = [max(0, round(len(fq) * wi / tot)) if fq else 0 for wi in w]
                fi = 0
                for i in range(CT):
                    n_i = i // 4 + 1
                    vw = (i + 1) * P          # causally-valid row width
                    if i < 4:   # short rows fit a 512-wide psq slot; using the
                        # other pool splits the exp-release chain between rows
                        rows = [psq.tile([P, 512], f32, tag="psq", name=f"r{k}_{i}")
                                for k in range(2)]
                    else:
                        rows = [pss.tile([P, 1024], f32, tag="srow", name=f"r{k}_{i}")
                                for k in range(2)]
                    for j in range(n_i):
                        ntrim = min(512, vw - j * 512)
                        for k, (a, off) in enumerate(heads):
                            nc.tensor.matmul(
                                rows[k][:, j * 512:j * 512 + ntrim],
                                K_T[off:off + H, p, i * P:(i + 1) * P],
                                Q_T[off:off + H, p, j * 512:j * 512 + ntrim],
                                start=True, stop=True,
                                skip_group_check=True,
                            )
                    if i in ACT_ACCUM_TILES:
                        # additive NEG mask on the diagonal block, then exp
                        # with the row sum accumulated on the scalar engine
                        for k, (a, off) in enumerate(heads):
                            nc.vector.tensor_tensor(
                                out=rows[k][:, i * P:(i + 1) * P],
                                in0=rows[k][:, i * P:(i + 1) * P],
                                in1=mskn[:],
                                op=add,
                            )
                            nc.scalar.activation(
                                Ets[k][:, _off(i):_off(i) + vw], rows[k][:, :vw],
                                Exp, accum_out=sums[k][:, i:i + 1],
                            )
                    else:
                        for k, (a, off) in enumerate(heads):
                            nc.scalar.activation(
                                Ets[k][:, _off(i):_off(i) + vw], rows[k][:, :vw],
                                Exp,
                            )
                            # fused: multiplicative 0/1 mask on the diagonal
                            # block and the causal row sum, one DVE pass
                            nc.vector.scalar_tensor_tensor(
                                out=Ets[k][:, _off(i):_off(i) + vw],
                                in0=Ets[k][:, _off(i):_off(i) + vw],
                                scalar=1.0,
                                in1=m1[:, S - i * P:S - i * P + vw],
                                op0=mult,
                                op1=mult,
                                accum_out=sums[k][:, i:i + 1],
                            )
                    if i < 4 and eq:
                        eq.pop(0)()
                    for _ in range(share[i]):
                        if fi < len(fq):
                            fq[fi]()
                            fi += 1
                while eq:
                    eq.pop(0)()
                while fi < len(fq):
                    fq[fi]()
                    fi += 1
                # normalization: batched reciprocal per head, then ONE
                # broadcast tensor_tensor per head scales all 8 V row-tiles
                for k, (a, off) in enumerate(heads):
                    nc.vector.reciprocal(rcps[k][:], sums[k][:])
                    nc.vector.tensor_tensor(
                        out=V[:, 0:CT, a * H:(a + 1) * H],
                        in0=V[:, 0:CT, a * H:(a + 1) * H],
                        in1=rcps[k][:, 0:CT, None].to_broadcast([P, CT, H]),
                        op=mult,
                    )
                return heads, Ets

            def av_closures(p, heads, Ets):
                """AV block of pair p as filler closures (descending i, causal
                N-trim, two heads col-split in one PSUM bank per chunk j)."""
                state = {}
                cs = []

                def step(j, i):
                    if i == CT - 1:
                        state[j] = psq.tile([P, 512], f32, tag="psq",
                                            name=f"za_{p}_{j}")
                    za = state[j]
                    ntrim = min(512, (i - 4 * j) * P + P)
                    for k, (a, off) in enumerate(heads):
                        nc.tensor.matmul(
                            za[off:off + H, :ntrim],
                            V[:, i, a * H:(a + 1) * H],
                            Ets[k][:, _off(i) + j * 512:_off(i) + j * 512 + ntrim],
                            start=(i == CT - 1), stop=(i == 4 * j),
                            skip_group_check=True,
                        )

                def copy(j):
                    nc.vector.tensor_copy(out=zT[:, p, j * 512:(j + 1) * 512],
                                          in_=state[j][:])

                for j in range(NC):
                    for i in range(CT - 1, 4 * j - 1, -1):
                        cs.append(lambda j=j, i=i: step(j, i))
                    cs.append(lambda j=j: copy(j))
                return cs

            # ---- output projection pieces ----
            def wo_partial(m, n_):
                """First 6 f-contraction steps of out-block (m, n_) into an
                fp16 partial (runs early, inside pairs 6/7)."""
                ps = psq.tile([P, 512], f32, tag="psq", name=f"wp{m}{n_}")
                for p2 in range(6):
                    nc.tensor.matmul(
                        ps[:],
                        zT[:, p2, m * P:(m + 1) * P],
                        wo[:, p2, n_ * 512:(n_ + 1) * 512],
                        start=(p2 == 0), stop=(p2 == 5),
                    )
                nc.vector.tensor_copy(out=wpart[:, m, n_, :], in_=ps[:])

            def wo_finish(m, n_, last=False):
                ps = psq.tile([P, 512], f32, tag="psq", name=f"wf{m}{n_}")
                for p2 in (6, 7):
                    nc.tensor.matmul(
                        ps[:],
                        zT[:, p2, m * P:(m + 1) * P],
                        wo[:, p2, n_ * 512:(n_ + 1) * 512],
                        start=(p2 == 6), stop=(p2 == 7),
                    )
                ot = outp.tile([P, 512], f16, tag="ot")
                nc.vector.tensor_tensor(out=ot[:], in0=ps[:],
                                        in1=wpart[:, m, n_, :], op=add)
                _out_dma(m, n_, ot, last)

            def wo_full(m, n_, last=False):
                ps = psq.tile([P, 512], f32, tag="psq", name=f"o{m}{n_}")
                for p2 in range(ET):
                    nc.tensor.matmul(
                        ps[:],
                        zT[:, p2, m * P:(m + 1) * P],
                        wo[:, p2, n_ * 512:(n_ + 1) * 512],
                        start=(p2 == 0), stop=(p2 == ET - 1),
                    )
                ot = outp.tile([P, 512], f16, tag="ot")
                nc.scalar.copy(out=ot[:], in_=ps[:])
                _out_dma(m, n_, ot, last)

            def _out_dma(m, n_, ot, last):
                nc.sync.dma_start(
                    out_d[m * P:(m + 1) * P, n_ * 512:(n_ + 1) * 512], ot[:])

            # ---- merged schedule ----
            for mat in (0, 1):
                kq_group(0, mat, 0)

            pair_early = {p: kq_cc(p, 1, "vector" if p % 4 == 3 else "scalar")
                          for p in range(8)}
            pair_fillers = {
                0: kq_cc(1, 0) + v_closures(0),
                1: kq_cc(2, 0),
                2: kq_cc(3, 0) + v_closures(1)[0:2],
                3: kq_cc(4, 0) + v_closures(1)[2:6],
                4: kq_cc(5, 0) + v_closures(1)[6:8],
                5: kq_cc(6, 0),
                6: kq_cc(7, 0) + [lambda m=m, n_=n_: wo_partial(m, n_)
                                  for m in (0, 1) for n_ in range(NC)],
                7: [lambda m=m, n_=n_: wo_partial(m, n_)
                    for m in (2, 3) for n_ in range(NC)],
            }
            av_prev = None
            for p in range(8):
                fillers = pair_fillers.get(p, [])
                if av_prev is not None:
                    fillers = av_prev + fillers
                heads, Ets = attn_rows(p, pair_early[p], fillers)
                av_prev = av_closures(p, heads, Ets)
            for cl in av_prev:             # AV of pair 7
                cl()

            # ---- output projection tail: full blocks m 4-7 first, then the
            # two-step finishes of the pre-accumulated blocks m 0-3 ----
            for m in range(4, CT):
                for n_ in range(NC):
                    wo_full(m, n_)
            for m in range(4):
                for n_ in range(NC):
                    wo_finish(m, n_, last=(m == 3 and n_ == NC - 1))

    # HW allows only one sync-wait per instruction (matmuls especially);
    # split excess waits into InstEventSemaphore like the bacc layer does.
    import bass_rust
    bass_rust.generate_event_semaphores(nc)
    return nc


def _host_prep(x, W_K, W_Q, W_V, W_O):
    """Pack per-core input dicts (host-side layout prep, fp16 casts)."""
    wk = W_K.transpose(2, 0, 1).reshape(E, A * H)
    wq = (W_Q / np.sqrt(H)).transpose(2, 0, 1).reshape(E, A * H)
    wv = W_V.transpose(2, 0, 1).reshape(E, A * H)
    wkqv = np.concatenate([wk, wq, wv], axis=1).astype(np.float16)
    wo = np.ascontiguousarray(W_O.T).astype(np.float16)

    r = np.arange(P)[:, None]
    d = np.arange(P)[None, :]
    # ident | ones-prefix + 0/1 diagonal block | additive NEG diagonal mask
    aux = np.concatenate(
        [np.eye(P, dtype=np.float16),
         np.ones((P, S), dtype=np.float16),
         np.where(d <= r, 1.0, 0.0).astype(np.float16),
         np.where(d <= r, 0.0, NEG).astype(np.float16)], axis=1)

    in_maps = []
    for b in range(B):
        in_maps.append({
            "xt": np.ascontiguousarray(x[b].T).astype(np.float16),
            "wkqv": wkqv,
            "wo": wo,
            "aux": aux,
        })
    return in_maps


def _run(x, W_K, W_Q, W_V, W_O, **spmd_kwargs):
    from concourse.bass_utils import run_bass_kernel_spmd

    if "nc" not in _cache:
        _cache["nc"] = _build_nc()
    in_maps = _host_prep(
        np.asarray(x, dtype=np.float32), np.asarray(W_K, dtype=np.float32),
        np.asarray(W_Q, dtype=np.float32), np.asarray(W_V, dtype=np.float32),
        np.asarray(W_O, dtype=np.float32),
    )
    res = run_bass_kernel_spmd(_cache["nc"], in_maps, core_ids=list(range(B)),
                               **spmd_kwargs)
    out = np.stack([r["out"] for r in res.results], axis=0).astype(np.float32)
    return out, res


def kernel(x, W_K, W_Q, W_V, W_O):
    out, _ = _run(x, W_K, W_Q, W_V, W_O)
    return out


# revision 18
# speedup vs baseline: 1.2576x; 1.2576x over previous
"""Trainium2 Bass kernel for 16-head causal attention (transposed-softmax variant).

Problem shapes: x [8, 1024, 1024]; W_K/W_Q/W_V [16, 64, 1024]; W_O [1024, 1024].
Sharding: pure data-parallel over batch (8 batch elements -> 8 cores), weights
replicated, no collectives.

Per-core pipeline (one batch element, seq=1024, d_embed=1024, 16 heads x 64):
  1. QKV projections as K_T/Q_T [heads*64, seq] and V [seq, heads*64], fp16
     operands, fp32 PSUM accumulation. W_Q is pre-scaled by 1/sqrt(d_head) on
     the host so scores come out pre-scaled.
  2. Per head pair: scores S[c, C] = K[c].Q[C] for causal-allowed C-chunks
     only, the two heads' K=64 matmuls interleaved so they run concurrently in
     disjoint PE row-groups.
  3. Softmax over C without max-subtraction (scores ~ N(0,1)). Row-tiles
     i<=5: exp on the scalar engine, then one fused vector-engine
     scalar_tensor_tensor per (head, tile): E *= m1-slice (ones over the
     strict prefix, lower-triangular 0/1 over the diagonal block) in place,
     with accum_out = causal row sum. Row-tiles i in {6, 7} (the widest, most
     expensive on the vector engine): additive NEG mask on the diagonal block
     (vector-engine add into PSUM), then exp WITH accum_out on the scalar
     engine — splitting the row-sum work across both engines. Reciprocals are
     batched per head ([128, 8] in one op); V rows are scaled by 1/rowsum with
     one broadcast tensor_tensor per head.
  4. Z^T[h, C] += V'^T E per c-tile (descending i, N trimmed to the causal
     prefix, two heads col-split in one PSUM bank); output projection
     Z_flat @ W_O^T. For output row-blocks m 0-3, the first six
     f-contraction steps run early (inside pairs 6/7, which are otherwise
     PE-lean) into an fp16 partial; the tail only finishes p2 in {6,7} and
     adds the partial, shortening the serial output phase.

Scheduling: the scalar engine carries ~115us of exp(+accum) and the PE
~145us of matmul; dense projection groups spread as fillers across ALL eight
pairs so the PE never starves and the HAM clock gate stays at full rate.
kq weights for pair p+1 cc=0 are emitted inside pair p; the cc=1 half is
emitted inside pair p+1's first rows (it is only needed from row 4 on),
which starts the first exp several microseconds earlier. Each pair's AV
block interleaves into the next pair's rows.
"""

import numpy as np

S, E, A, H, B = 1024, 1024, 16, 64, 8
P = 128          # partitions
NEG = -30000.0   # additive mask value (fp16-safe; exp -> 0 in fp32)

_cache = {}


def _off(i):
    """Compact E-buffer offset of row-tile i (valid width of row i is (i+1)*P)."""
    return P * i * (i + 1) // 2


EW = _off(8)     # 4608 columns total

ACT_ACCUM_TILES = (6, 7)   # row-tiles whose row sum rides the scalar engine


def _build_nc():
    import concourse.bass as bass
    import concourse.mybir as mybir
    from concourse.tile import TileContext

    f16 = mybir.dt.float16
    f32 = mybir.dt.float32
    Exp = mybir.ActivationFunctionType.Exp
    mult = mybir.AluOpType.mult
    add = mybir.AluOpType.add

    nc = bass.Bass()
    xt_d = nc.dram_tensor("xt", [E, S], f16, kind="ExternalInput")        # x[b].T
    wkqv_d = nc.dram_tensor("wkqv", [E, 3 * A * H], f16, kind="ExternalInput")
    wo_d = nc.dram_tensor("wo", [A * H, E], f16, kind="ExternalInput")    # W_O.T
    aux_d = nc.dram_tensor("aux", [P, P + (S + P) + P], f16,
                           kind="ExternalInput")  # ident | m1 | mskn packed
    out_d = nc.dram_tensor("out", [S, E], f16, kind="ExternalOutput")

    ET = E // P       # 8 e-tiles
    CT = S // P       # 8 c-tiles
    NC = S // 512     # 2 512-chunks

    with TileContext(nc) as tc:
        with (
            tc.tile_pool(name="inp", bufs=1) as inp,
            tc.tile_pool(name="kqv", bufs=1) as kqv,
            tc.tile_pool(name="epool", bufs=4) as epool,
            tc.tile_pool(name="stats", bufs=12) as stats,
            tc.tile_pool(name="outp", bufs=3) as outp,
            tc.tile_pool(name="psq", bufs=4, space="PSUM") as psq,
            tc.tile_pool(name="pss", bufs=2, space="PSUM") as pss,
        ):
            # ---- SBUF destinations ----
            xT = inp.tile([P, ET, S], f16, tag="xT")
            wkqv = inp.tile([P, ET, 3 * A * H], f16, tag="wkqv")
            wo = inp.tile([P, ET, E], f16, tag="wo")
            aux = inp.tile([P, P + (S + P) + P], f16, tag="aux")
            wpart = inp.tile([P, 4, NC, 512], f16, tag="wpart")
            ident = aux[:, 0:P]
            m1 = aux[:, P:P + S + P]
            mskn = aux[:, P + S + P:P + S + 2 * P]

            # ---- loads: ONE queue so queue order = priority order (parallel
            # queues share SDMA bandwidth and starve the critical transfers).
            # et-pair granularity up front for fine deps, batched tails. ----
            xr = xt_d[:].rearrange("(t p) c -> p t c", p=P)      # [128, 8, 1024]
            wr = wkqv_d[:].rearrange("(t p) f -> p t f", p=P)    # [128, 8, 3072]
            wor = wo_d[:].rearrange("(t p) f -> p t f", p=P)
            nc.sync.dma_start(aux[:], aux_d[:])
            for t2 in range(0, ET, 2):   # pair-0 K+Q cols and first x half
                sl = slice(t2, t2 + 2)
                nc.sync.dma_start(wkqv[:, sl, 0:P], wr[:, sl, 0:P])
                nc.sync.dma_start(wkqv[:, sl, A * H:A * H + P],
                                  wr[:, sl, A * H:A * H + P])
                nc.sync.dma_start(xT[:, sl, 0:512], xr[:, sl, 0:512])
            for t4 in range(0, ET, 4):   # second x half
                sl = slice(t4, t4 + 4)
                nc.sync.dma_start(xT[:, sl, 512:S], xr[:, sl, 512:S])
            for t4 in range(0, ET, 4):   # K rest (kq(1..) fillers)
                sl = slice(t4, t4 + 4)
                nc.sync.dma_start(wkqv[:, sl, P:A * H], wr[:, sl, P:A * H])
            for t4 in range(0, ET, 4):   # V fc0 (pair 0-3 v-groups)
                sl = slice(t4, t4 + 4)
                nc.sync.dma_start(wkqv[:, sl, 2 * A * H:2 * A * H + 512],
                                  wr[:, sl, 2 * A * H:2 * A * H + 512])
            for t4 in range(0, ET, 4):   # Q rest
                sl = slice(t4, t4 + 4)
                nc.sync.dma_start(wkqv[:, sl, A * H + P:2 * A * H],
                                  wr[:, sl, A * H + P:2 * A * H])
            for t4 in range(0, ET, 4):   # V fc1
                sl = slice(t4, t4 + 4)
                nc.sync.dma_start(wkqv[:, sl, 2 * A * H + 512:3 * A * H],
                                  wr[:, sl, 2 * A * H + 512:3 * A * H])
            for t4 in range(0, ET, 4):   # output projection weights
                sl = slice(t4, t4 + 4)
                nc.sync.dma_start(wo[:, sl, :], wor[:, sl, :])

            K_T = kqv.tile([P, A // 2, S], f16, tag="K_T")   # pair-stacked [2h, c]
            Q_T = kqv.tile([P, A // 2, S], f16, tag="Q_T")
            V = kqv.tile([P, CT, A * H], f16, tag="V")       # [c, f]
            zT = kqv.tile([P, A // 2, S], f16, tag="zT")     # pair-stacked [f, C]

            # ---- PE warm-up: ride out the HAM throttle during the DMA wait ----
            wps = psq.tile([P, 512], f32, tag="psq", name="warm")
            for w in range(28):
                nc.tensor.matmul(wps[:, :P], ident[:], ident[:],
                                 start=(w == 0), stop=(w == 27),
                                 skip_group_check=True)
            wsb = stats.tile([P, 1], f32, tag="ssum", name="warmsink")
            nc.vector.reduce_max(wsb[:], wps[:, :P], axis=mybir.AxisListType.X)

            # ---- dense projection groups (used as attention fillers) ----
            def kq_group(p, mat, cc, cast_eng="scalar"):
                dst = K_T if mat == 0 else Q_T
                ps = psq.tile([P, 512], f32, tag="psq", name=f"q{p}{mat}{cc}")
                for et in range(ET):
                    nc.tensor.matmul(
                        ps[:],
                        wkqv[:, et, mat * A * H + p * P: mat * A * H + (p + 1) * P],
                        xT[:, et, cc * 512:(cc + 1) * 512],
                        start=(et == 0), stop=(et == ET - 1),
                    )
                if cast_eng == "scalar":
                    nc.scalar.copy(out=dst[:, p, cc * 512:(cc + 1) * 512], in_=ps[:])
                else:
                    nc.vector.tensor_copy(out=dst[:, p, cc * 512:(cc + 1) * 512],
                                          in_=ps[:])

            def v_group(fc, i):
                ps = psq.tile([P, 512], f32, tag="psq", name=f"v{fc}{i}")
                for et in range(ET):
                    nc.tensor.matmul(
                        ps[:],
                        xT[:, et, i * P:(i + 1) * P],
                        wkqv[:, et, 2 * A * H + fc * 512: 2 * A * H + (fc + 1) * 512],
                        start=(et == 0), stop=(et == ET - 1),
                    )
                nc.vector.tensor_copy(out=V[:, i, fc * 512:(fc + 1) * 512], in_=ps[:])

            def kq_cc(p, cc, cast_eng="scalar"):
                return [lambda mat=mat: kq_group(p, mat, cc, cast_eng)
                        for mat in (0, 1)]

            def v_closures(fc):
                return [lambda fc=fc, i=i: v_group(fc, i) for i in range(CT)]

            # ---- attention ----
            def attn_rows(p, early, fillers):
                """Scores+exp+mask/rowsum rows of pair p. `early` closures run
                in rows 0-3 (cc=1 kq groups, needed from row 4); `fillers`
                interleave across all rows. Returns (heads, E tiles)."""
                heads = [(2 * p, 0), (2 * p + 1, H)]
                Ets = [epool.tile([P, EW], f16, tag="E", name=f"E{k}_{p}")
                       for k in range(2)]
                sums = [stats.tile([P, CT], f32, tag="ssum", name=f"sm{k}_{p}")
                        for k in range(2)]
                rcps = [stats.tile([P, CT], f32, tag="rcp", name=f"rc{k}_{p}")
                        for k in range(2)]
                fq = list(fillers)
                eq = list(early)
                w = [2, 2, 2, 2, 3, 3, 3, 3]
                tot = sum(w)
                share = [max(0, round(len(fq) * wi / tot)) if fq else 0 for wi in w]
                fi = 0
                for i in range(CT):
                    n_i = i // 4 + 1
                    vw = (i + 1) * P          # causally-valid row width
                    if i < 4:   # short rows fit a 512-wide psq slot; using the
                        # other pool splits the exp-release chain between rows
                        rows = [psq.tile([P, 512], f32, tag="psq", name=f"r{k}_{i}")
                                for k in range(2)]
                    else:
                        rows = [pss.tile([P, 1024], f32, tag="srow", name=f"r{k}_{i}")
                                for k in range(2)]
                    for j in range(n_i):
                        ntrim = min(512, vw - j * 512)
                        for k, (a, off) in enumerate(heads):
                            nc.tensor.matmul(
                                rows[k][:, j * 512:j * 512 + ntrim],
                                K_T[off:off + H, p, i * P:(i + 1) * P],
                                Q_T[off:off + H, p, j * 512:j * 512 + ntrim],
                                start=True, stop=True,
                                skip_group_check=True,
                            )
                    if i in ACT_ACCUM_TILES:
                        # additive NEG mask on the diagonal block, then exp
                        # with the row sum accumulated on the scalar engine
                        for k, (a, off) in enumerate(heads):
                            nc.vector.tensor_tensor(
                                out=rows[k][:, i * P:(i + 1) * P],
                                in0=rows[k][:, i * P:(i + 1) * P],
                                in1=mskn[:],
                                op=add,
                            )
                            nc.scalar.activation(
                                Ets[k][:, _off(i):_off(i) + vw], rows[k][:, :vw],
                                Exp, accum_out=sums[k][:, i:i + 1],
                            )
                    else:
                        for k, (a, off) in enumerate(heads):
                            nc.scalar.activation(
                                Ets[k][:, _off(i):_off(i) + vw], rows[k][:, :vw],
                                Exp,
                            )
                            # fused: multiplicative 0/1 mask on the diagonal
                            # block and the causal row sum, one DVE pass
                            nc.vector.scalar_tensor_tensor(
                                out=Ets[k][:, _off(i):_off(i) + vw],
                                in0=Ets[k][:, _off(i):_off(i) + vw],
                                scalar=1.0,
                                in1=m1[:, S - i * P:S - i * P + vw],
                                op0=mult,
                                op1=mult,
                                accum_out=sums[k][:, i:i + 1],
                            )
                    if i < 4 and eq:
                        eq.pop(0)()
                    for _ in range(share[i]):
                        if fi < len(fq):
                            fq[fi]()
                            fi += 1
                while eq:
                    eq.pop(0)()
                while fi < len(fq):
                    fq[fi]()
                    fi += 1
                # normalization: batched reciprocal per head, then ONE
                # broadcast tensor_tensor per head scales all 8 V row-tiles
                for k, (a, off) in enumerate(heads):
                    nc.vector.reciprocal(rcps[k][:], sums[k][:])
                    nc.vector.tensor_tensor(
                        out=V[:, 0:CT, a * H:(a + 1) * H],
                        in0=V[:, 0:CT, a * H:(a + 1) * H],
                        in1=rcps[k][:, 0:CT, None].to_broadcast([P, CT, H]),
                        op=mult,
                    )
                return heads, Ets

            def av_closures(p, heads, Ets):
                """AV block of pair p as filler closures (descending i, causal
                N-trim, two heads col-split in one PSUM bank per chunk j)."""
                state = {}
                cs = []

                def step(j, i):
                    if i == CT - 1:
                        state[j] = psq.tile([P, 512], f32, tag="psq",
                                            name=f"za_{p}_{j}")
                    za = state[j]
                    ntrim = min(512, (i - 4 * j) * P + P)
                    for k, (a, off) in enumerate(heads):
                        nc.tensor.matmul(
                            za[off:off + H, :ntrim],
                            V[:, i, a * H:(a + 1) * H],
                            Ets[k][:, _off(i) + j * 512:_off(i) + j * 512 + ntrim],
                            start=(i == CT - 1), stop=(i == 4 * j),
                            skip_group_check=True,
                        )

                def copy(j):
                    nc.vector.tensor_copy(out=zT[:, p, j * 512:(j + 1) * 512],
                                          in_=state[j][:])

                for j in range(NC):
                    for i in range(CT - 1, 4 * j - 1, -1):
                        cs.append(lambda j=j, i=i: step(j, i))
                    cs.append(lambda j=j: copy(j))
                return cs

            # ---- output projection pieces ----
            def wo_partial(m, n_):
                """First 6 f-contraction steps of out-block (m, n_) into an
                fp16 partial (runs early, inside pairs 6/7)."""
                ps = psq.tile([P, 512], f32, tag="psq", name=f"wp{m}{n_}")
                for p2 in range(6):
                    nc.tensor.matmul(
                        ps[:],
                        zT[:, p2, m * P:(m + 1) * P],
                        wo[:, p2, n_ * 512:(n_ + 1) * 512],
                        start=(p2 == 0), stop=(p2 == 5),
                    )
                nc.vector.tensor_copy(out=wpart[:, m, n_, :], in_=ps[:])

            def wo_finish(m, n_, last=False):
                ps = psq.tile([P, 512], f32, tag="psq", name=f"wf{m}{n_}")
                for p2 in (6, 7):
                    nc.tensor.matmul(
                        ps[:],
                        zT[:, p2, m * P:(m + 1) * P],
                        wo[:, p2, n_ * 512:(n_ + 1) * 512],
                        start=(p2 == 6), stop=(p2 == 7),
                    )
                ot = outp.tile([P, 512], f16, tag="ot")
                nc.vector.tensor_tensor(out=ot[:], in0=ps[:],
                                        in1=wpart[:, m, n_, :], op=add)
                _out_dma(m, n_, ot, last)

            def wo_full(m, n_, last=False):
                ps = psq.tile([P, 512], f32, tag="psq", name=f"o{m}{n_}")
                for p2 in range(ET):
                    nc.tensor.matmul(
                        ps[:],
                        zT[:, p2, m * P:(m + 1) * P],
                        wo[:, p2, n_ * 512:(n_ + 1) * 512],
                        start=(p2 == 0), stop=(p2 == ET - 1),
                    )
                ot = outp.tile([P, 512], f16, tag="ot")
                nc.scalar.copy(out=ot[:], in_=ps[:])
                _out_dma(m, n_, ot, last)

            def _out_dma(m, n_, ot, last):
                nc.sync.dma_start(
                    out_d[m * P:(m + 1) * P, n_ * 512:(n_ + 1) * 512], ot[:])

            # ---- merged schedule ----
            for mat in (0, 1):
                kq_group(0, mat, 0)

            pair_early = {p: kq_cc(p, 1, "vector" if p % 4 == 3 else "scalar")
                          for p in range(8)}
            pair_fillers = {
                0: kq_cc(1, 0) + v_closures(0),
                1: kq_cc(2, 0),
                2: kq_cc(3, 0) + v_closures(1)[0:2],
                3: kq_cc(4, 0) + v_closures(1)[2:6],
                4: kq_cc(5, 0) + v_closures(1)[6:8],
                5: kq_cc(6, 0),
                6: kq_cc(7, 0) + [lambda m=m, n_=n_: wo_partial(m, n_)
                                  for m in (0, 1) for n_ in range(NC)],
                7: [lambda m=m, n_=n_: wo_partial(m, n_)
                    for m in (2, 3) for n_ in range(NC)],
            }
            av_prev = None
            for p in range(8):
                fillers = pair_fillers.get(p, [])
                if av_prev is not None:
                    fillers = av_prev + fillers
                heads, Ets = attn_rows(p, pair_early[p], fillers)
                av_prev = av_closures(p, heads, Ets)
            for cl in av_prev:             # AV of pair 7
                cl()

            # ---- output projection tail: full blocks m 4-7 first, then the
            # two-step finishes of the pre-accumulated blocks m 0-3 ----
            for m in range(4, CT):
                for n_ in range(NC):
                    wo_full(m, n_)
            for m in range(4):
                for n_ in range(NC):
                    wo_finish(m, n_, last=(m == 3 and n_ == NC - 1))

    # HW allows only one sync-wait per instruction (matmuls especially);
    # split excess waits into InstEventSemaphore like the bacc layer does.
    import bass_rust
    bass_rust.generate_event_semaphores(nc)
    return nc


def _host_prep(x, W_K, W_Q, W_V, W_O):
    """Pack per-core input dicts (host-side layout prep, fp16 casts)."""
    wk = W_K.transpose(2, 0, 1).reshape(E, A * H)
    wq = (W_Q / np.sqrt(H)).transpose(2, 0, 1).reshape(E, A * H)
    wv = W_V.transpose(2, 0, 1).reshape(E, A * H)
    wkqv = np.concatenate([wk, wq, wv], axis=1).astype(np.float16)
    wo = np.ascontiguousarray(W_O.T).astype(np.float16)

    r = np.arange(P)[:, None]
    d = np.arange(P)[None, :]
    # ident | ones-prefix + 0/1 diagonal block | additive NEG diagonal mask
    aux = np.concatenate(
        [np.eye(P, dtype=np.float16),
         np.ones((P, S), dtype=np.float16),
         np.where(d <= r, 1.0, 0.0).astype(np.float16),
         np.where(d <= r, 0.0, NEG).astype(np.float16)], axis=1)

    in_maps = []
    for b in range(B):
        in_maps.append({
            "xt": np.ascontiguousarray(x[b].T).astype(np.float16),
            "wkqv": wkqv,
            "wo": wo,
            "aux": aux,
        })
    return in_maps


def _run(x, W_K, W_Q, W_V, W_O, **spmd_kwargs):
    from concourse.bass_utils import run_bass_kernel_spmd

    if "nc" not in _cache:
        _cache["nc"] = _build_nc()
    in_maps = _host_prep(
        np.asarray(x, dtype=np.float32), np.asarray(W_K, dtype=np.float32),
        np.asarray(W_Q, dtype=np.float32), np.asarray(W_V, dtype=np.float32),
        np.asarray(W_O, dtype=np.float32),
    )
    res = run_bass_kernel_spmd(_cache["nc"], in_maps, core_ids=list(range(B)),
                               **spmd_kwargs)
    out = np.stack([r["out"] for r in res.results], axis=0).astype(np.float32)
    return out, res


def kernel(x, W_K, W_Q, W_V, W_O):
    out, _ = _run(x, W_K, W_Q, W_V, W_O)
    return out
